# revision 1
# baseline (speedup 1.0000x reference)
"""AlphaFold3Loss Bass kernel for 8 TRN2 NeuronCores.

Sharding: distogram rows (768 -> 96/core), LDDT atom-rows (3072 -> 384/core),
MSE reductions replicated-free (each core does its own row range... core 0 only
actually -- tiny). Device computes all O(N^2) work; host combines scalar
partials (incl. 3x3 SVD for the Kabsch trace term).

Distogram math per pair (i,j):
  err = logsumexp_b(L_b) - L_tb,  tb = #(bounds < d2_ij)
  L_tb = L_0 + sum_b 1[bound_b < d2] * (L_{b+1} - L_b)   (telescoping)
Only the global sum of err is needed -> per-partition accumulators.

LDDT per pair: dp/dg from PE K=5 augmented matmuls (d2 = rn_m + rn_n - 2<x,x>
+ d_eps in one matmul), delta' = max(|dp-dg|, 30*(dg>=15)) so sigmoid terms of
cutoff-masked pairs vanish; sigmoid sums via ACT accum_out. Diagonal pairs are
included on device and subtracted on host.

Assumes token_mask/atom_exists are all ones (they are, per setup_inputs);
if not, kernel() falls back to an exact numpy path.
"""
import sys
sys.path.insert(0, '/opt/trn_rl_repo')
import numpy as np
from contextlib import ExitStack

NT, NO_BINS, NA = 768, 64, 3072
NCORES = 8
RPC = NT // NCORES          # 96 distogram rows per core
APC = NA // NCORES          # 384 lddt atom-rows per core
NMT = RPC // 8              # 12 mega-tiles (8 rows each)
D_EPS = 4e-3                # lddt sqrt guard (host-corrected; see module doc)
BIG = 30.0

# cpak column layout
C_BOUNDS = 0                # 378 = 6*63
C_D2T = 378                 # 576 = 6*96  (d2t[p, k, i] = d2[row_i, 6p+k])
C_BIAS = 954                # 4 sigmoid biases
C_PRED = 958                # 72
C_PREDM = 1030              # 72
C_GT = 1102                 # 72
C_GTM = 1174                # 72
C_MW = 1246                 # 24
CW = 1280
# pgx column layout (partition dim = 7, fp16): A/B aug forms, 40 tiles x 128
NTIL = 40                   # 36 offdiag (w=2) + 4 diag (w=1) lddt tiles/core
P_PA, P_PB, P_GA, P_GB = 0, 5120, 10240, 15360
PGW = 20480
# out column layout
O_S = 0                     # 576 per-pair sum-exp
O_B = 576                   # 12 ind*g sums
O_L0 = 588                  # 12 L0 sums
O_CC = 600                  # 10 c counts (per lddt group)
O_SIG = 612                 # 8 sigmoid accums (4 k x {offdiag, diag})
O_MSE = 620                 # 18: Sw, A3(3), B3(3), M9(9), Spp, Sgg
OW = 640

_cache = {}


def _build_graph(phases=("disto", "lddt", "mse"), reps=1):
    from concourse import bass, bacc, tile, mybir
    F32 = mybir.dt.float32
    F16 = mybir.dt.float16
    BF16 = mybir.dt.bfloat16
    U32 = mybir.dt.uint32
    AF = mybir.ActivationFunctionType
    ALU = mybir.AluOpType
    AX = mybir.AxisListType

    from concourse.tile import add_dep_helper
    nc = bacc.Bacc(None, target_bir_lowering=False)
    lg_ext = nc.declare_dram_parameter("logits", [128, RPC, 384], F32, isOutput=False)
    cp_ext = nc.declare_dram_parameter("cpak", [128, CW], F32, isOutput=False)
    px_ext = nc.declare_dram_parameter("pgx", [7, PGW], F16, isOutput=False)
    out_ext = nc.declare_dram_parameter("out", [128, OW], F32, isOutput=True)

    with tile.TileContext(nc) as tc, ExitStack() as ctx:
        const = ctx.enter_context(tc.tile_pool(name="const", bufs=1))
        lpool = ctx.enter_context(tc.tile_pool(name="lp", bufs=3))
        epool = ctx.enter_context(tc.tile_pool(name="ep", bufs=2))
        wpool = ctx.enter_context(tc.tile_pool(name="wp", bufs=2))
        spool = ctx.enter_context(tc.tile_pool(name="sp", bufs=2))
        psum = ctx.enter_context(tc.tile_pool(name="ps", bufs=1, space="PSUM"))

        cpak = const.tile([128, CW], F32)
        nc.sync.dma_start(cpak[:], cp_ext[:, :])
        pgx = const.tile([7, PGW], F16)
        nc.sync.dma_start(pgx[:], px_ext[:, :])
        outb = const.tile([128, OW], F32)
        nc.vector.memset(outb[:], 0.0)
        if reps > 1:
            racc = const.tile([128, OW], F32)
            nc.vector.memset(racc[:], 0.0)
        dpr = const.tile([128, 10 * 512], BF16)   # stored |delta'| for phase B

        exp_insts, sqrt_insts, sig_insts = [], [], []

        for _rep in range(reps):
          if True:
            # ---------------- LDDT phase A: matmuls + sqrt + delta' ----------
            if "lddt" in phases:
                for g10 in range(10):
                    psP = psum.tile([128, 512], F32, tag="psP", bufs=2)
                    psG = psum.tile([128, 512], F32, tag="psG", bufs=2)
                    for q in range(4):
                        t = g10 * 4 + q
                        nc.tensor.matmul(psP[:, q * 128:(q + 1) * 128],
                                         lhsT=pgx[:, P_PA + t * 128:P_PA + (t + 1) * 128],
                                         rhs=pgx[:, P_PB + t * 128:P_PB + (t + 1) * 128],
                                         start=True, stop=True)
                        nc.tensor.matmul(psG[:, q * 128:(q + 1) * 128],
                                         lhsT=pgx[:, P_GA + t * 128:P_GA + (t + 1) * 128],
                                         rhs=pgx[:, P_GB + t * 128:P_GB + (t + 1) * 128],
                                         start=True, stop=True)
                    dp = spool.tile([128, 512], F32, tag="dp")
                    sqrt_insts.append(nc.scalar.activation(dp[:], psP[:], AF.Sqrt))
                    dg = spool.tile([128, 512], F32, tag="dg")
                    sqrt_insts.append(nc.scalar.activation(dg[:], psG[:], AF.Sqrt))
                    delta = spool.tile([128, 512], F32, tag="delta")
                    nc.vector.tensor_sub(delta[:], dp[:], dg[:])
                    nc.vector.tensor_scalar(delta[:].bitcast(U32), delta[:].bitcast(U32),
                                            0x7FFFFFFF, None, ALU.bitwise_and)
                    cbar = spool.tile([128, 512], F32, tag="cbar")
                    nc.vector.tensor_scalar(cbar[:], dg[:], 15.0, BIG, ALU.is_ge, ALU.mult)
                    nc.vector.tensor_tensor(dpr[:, g10 * 512:(g10 + 1) * 512], delta[:], cbar[:], ALU.max)
                    # far-pair count from cbar: #(dg<15) = 512 - sum(cbar)/30 (host)
                    nc.vector.tensor_reduce(outb[:, O_CC + g10:O_CC + g10 + 1], cbar[:], AX.X, ALU.add)

            # ---------------- distogram ------------------------------------
            if "disto" in phases:
                bounds4 = cpak[:, C_BOUNDS:C_BOUNDS + 378].rearrange(
                    "p (k b) -> p k b", k=6).unsqueeze(1).broadcast_to([128, 8, 6, 63])
                d2t = cpak[:, C_D2T:C_D2T + 576].rearrange("p (k i) -> p k i", k=6)
                ones128 = const.tile([128, 1], BF16, tag="ones128")
                nc.vector.memset(ones128[:], 1.0)
                bacc_ps = psum.tile([1, 504], F32, tag="baccps", bufs=1)
                for mt in range(NMT):
                    L = lpool.tile([128, 8 * 384], F32, tag="L")
                    nc.sync.dma_start(L[:], lg_ext[:, 8 * mt:8 * mt + 8, :])
                    L4 = L[:].rearrange("p (r k b) -> p r k b", r=8, k=6)
                    E = epool.tile([128, 8 * 384], BF16, tag="E")
                    exp_insts.append(nc.scalar.activation(E[:], L[:], AF.Exp))
                    E3 = E[:].rearrange("p (a b) -> p a b", b=64)
                    w = 32
                    while w >= 1:
                        nc.vector.tensor_add(E3[:, :, 0:w], E3[:, :, 0:w], E3[:, :, w:2 * w])
                        w //= 2
                    nc.vector.tensor_copy(outb[:, O_S + mt * 48:O_S + (mt + 1) * 48],
                                          E3[:, :, 0])
                    Lb = epool.tile([128, 8 * 384], BF16, tag="Lb")
                    nc.scalar.copy(Lb[:], L[:])
                    Lb4 = Lb[:].rearrange("p (r k b) -> p r k b", r=8, k=6)
                    ind = wpool.tile([128, 8 * 378], BF16, tag="ind")
                    ind4 = ind[:].rearrange("p (r k b) -> p r k b", r=8, k=6)
                    d2b = d2t[:, :, 8 * mt:8 * mt + 8].rearrange("p k i -> p i k") \
                        .unsqueeze(3).broadcast_to([128, 8, 6, 63])
                    nc.vector.tensor_tensor(ind4, bounds4, d2b, ALU.is_lt)
                    g = wpool.tile([128, 8 * 378], BF16, tag="g")
                    g4 = g[:].rearrange("p (r k b) -> p r k b", r=8, k=6)
                    nc.vector.tensor_tensor(g4, Lb4[:, :, :, 1:64], Lb4[:, :, :, 0:63], ALU.subtract)
                    nc.vector.tensor_mul(g[:], ind[:], g[:])
                    for c6 in range(6):
                        nc.tensor.matmul(bacc_ps[:], lhsT=ones128[:],
                                         rhs=g[:, c6 * 504:(c6 + 1) * 504],
                                         start=(mt == 0 and c6 == 0),
                                         stop=(mt == NMT - 1 and c6 == 5),
                                         skip_group_check=True)
                    nc.vector.tensor_reduce(outb[:, O_L0 + mt:O_L0 + mt + 1],
                                            L4[:, :, :, 0], AX.XY, ALU.add)
                nc.vector.tensor_reduce(outb[0:1, O_B:O_B + 1], bacc_ps[:], AX.X, ALU.add)

            # ---------------- LDDT phase B: sigmoid sums ---------------------
            if "lddt" in phases:
                for part, lo, hi in ((0, 0, 9 * 512), (1, 9 * 512, 10 * 512)):
                    for k in range(4):
                        sg = spool.tile([128, 9 * 512], BF16, tag="sg")
                        sac = spool.tile([128, 1], F32, tag="sac")
                        sig_insts.append(nc.scalar.activation(
                            sg[:, 0:hi - lo], dpr[:, lo:hi], AF.Sigmoid,
                            bias=cpak[:, C_BIAS + k:C_BIAS + k + 1], scale=-1.0,
                            accum_out=sac[:]))
                        col = O_SIG + part * 4 + k
                        nc.vector.tensor_copy(outb[:, col:col + 1], sac[:])

            # ---------------- MSE reductions (tiny) --------------------------
            if "mse" in phases:
                def v3(col):
                    return cpak[:, col:col + 72].rearrange("p (a c) -> p a c", c=3)
                pred, predm, gt, gtm = v3(C_PRED), v3(C_PREDM), v3(C_GT), v3(C_GTM)
                mw = cpak[:, C_MW:C_MW + 24]
                t24 = wpool.tile([128, 24], F32, tag="t24")
                nc.vector.tensor_reduce(outb[:, O_MSE:O_MSE + 1], mw, AX.X, ALU.add)  # Sw
                for i in range(3):  # A3 = sum mw*gt, B3 = sum mw*pred
                    nc.vector.tensor_reduce(outb[:, O_MSE + 1 + i:O_MSE + 2 + i], gtm[:, :, i], AX.X, ALU.add)
                    nc.vector.tensor_reduce(outb[:, O_MSE + 4 + i:O_MSE + 5 + i], predm[:, :, i], AX.X, ALU.add)
                for i in range(3):  # M9[i,j] = sum mw*pred_i*gt_j
                    for j in range(3):
                        nc.vector.tensor_mul(t24[:], predm[:, :, i], gt[:, :, j])
                        c = O_MSE + 7 + 3 * i + j
                        nc.vector.tensor_reduce(outb[:, c:c + 1], t24[:], AX.X, ALU.add)
                acc1 = wpool.tile([128, 3], F32, tag="acc1")
                for i in range(3):  # Spp partial per coord
                    nc.vector.tensor_mul(t24[:], predm[:, :, i], pred[:, :, i])
                    nc.vector.tensor_reduce(acc1[:, i:i + 1], t24[:], AX.X, ALU.add)
                nc.vector.tensor_reduce(outb[:, O_MSE + 16:O_MSE + 17], acc1[:], AX.X, ALU.add)
                for i in range(3):  # Sgg
                    nc.vector.tensor_mul(t24[:], gtm[:, :, i], gt[:, :, i])
                    nc.vector.tensor_reduce(acc1[:, i:i + 1], t24[:], AX.X, ALU.add)
                nc.vector.tensor_reduce(outb[:, O_MSE + 17:O_MSE + 18], acc1[:], AX.X, ALU.add)

          if reps > 1:
            nc.vector.tensor_add(racc[:], racc[:], outb[:])

        # ACT-stream phase ordering: sqrt set -> exp set -> sigmoid set
        if sqrt_insts and exp_insts:
            add_dep_helper(sqrt_insts[-1].ins, exp_insts[0].ins, sync=False,
                           reason="act table: sqrts before exps")
        if exp_insts and sig_insts:
            add_dep_helper(exp_insts[-1].ins, sig_insts[0].ins, sync=False,
                           reason="act table: exps before sigmoids")
        elif sqrt_insts and sig_insts:
            add_dep_helper(sqrt_insts[-1].ins, sig_insts[0].ins, sync=False,
                           reason="act table: sqrts before sigmoids")

        nc.sync.dma_start(out_ext[:, :], racc[:] if reps > 1 else outb[:])
    nc.finalize()
    return nc


def _host_prep(inputs):
    lg = np.ascontiguousarray(inputs["distogram_logits"][0], dtype=np.float32)  # [768,768,64]
    pos = np.asarray(inputs["all_atom_positions"][0], dtype=np.float32)
    pred = np.asarray(inputs["denoised_atoms"][0], dtype=np.float32)            # [3072,3]
    gt = np.asarray(inputs["augmented_gt_atoms"][0], dtype=np.float32)
    ae = np.asarray(inputs["atom_exists"][0], dtype=np.float32)

    pb = pos[:, 1, :]                                        # CA positions [768,3]
    diff = pb[:, None, :] - pb[None, :, :]
    d2 = np.einsum('ijk,ijk->ij', diff, diff).astype(np.float32)   # [768,768]

    bounds63 = (np.linspace(0.0, 32.0, 63) ** 2).astype(np.float32)
    bounds378 = np.tile(bounds63, 6)

    def aug7(x):
        """fp16 K=7 aug: A (stationary) and B (moving) forms per atom.
        d2 = -2<xq,yq> + (hi_m+lo_m) + (hi_n+lo_n); rn from the fp16-quantized
        coords, hi/lo split so fp16 carries rn to ~1e-3 abs."""
        xq = x.astype(np.float16).astype(np.float64)
        rn = (xq ** 2).sum(-1)
        hi = rn.astype(np.float16)
        lo = rn - hi.astype(np.float64)
        one = np.ones(len(x))
        A = np.stack([-2 * xq[:, 0], -2 * xq[:, 1], -2 * xq[:, 2],
                      hi.astype(np.float64), lo + D_EPS, one, one]).astype(np.float16)
        B = np.stack([xq[:, 0], xq[:, 1], xq[:, 2], one, one,
                      hi.astype(np.float64), lo]).astype(np.float16)
        return A, B

    pA, pB = aug7(pred)
    gA, gB = aug7(gt)
    dumA = np.zeros((7, 128), np.float16)
    dumA[3] = 6.0e4; dumA[5] = 1.0; dumA[6] = 1.0
    dumB = np.zeros((7, 128), np.float16)
    dumB[3] = 1.0; dumB[4] = 1.0; dumB[5] = 6.0e4
    # symmetric block-tile assignment: 24 atom blocks of 128
    offd = [(i, j) for i in range(24) for j in range(i + 1, 24)]
    diag = [(i, i) for i in range(24)]
    mw = (ae * ae).astype(np.float32)
    predm = (pred * mw[:, None]).astype(np.float32)
    gtm = (gt * mw[:, None]).astype(np.float32)

    in_maps = []
    for c in range(NCORES):
        rows = slice(RPC * c, RPC * (c + 1))
        lgc = lg[rows].reshape(RPC, 128, 384).transpose(1, 0, 2)
        lgc = np.ascontiguousarray(lgc)
        cpak = np.zeros((128, CW), np.float32)
        cpak[:, C_BOUNDS:C_BOUNDS + 378] = bounds378
        # d2t[p, k, i] = d2[row_i, 6p+k]
        d2c = d2[rows].T.reshape(128, 6, RPC)
        cpak[:, C_D2T:C_D2T + 576] = d2c.reshape(128, 576)
        cpak[:, C_BIAS:C_BIAS + 4] = np.array([0.5, 1.0, 2.0, 4.0], np.float32)
        cpak[:, C_PRED:C_PRED + 72] = pred.reshape(128, 72)
        cpak[:, C_PREDM:C_PREDM + 72] = predm.reshape(128, 72)
        cpak[:, C_GT:C_GT + 72] = gt.reshape(128, 72)
        cpak[:, C_GTM:C_GTM + 72] = gtm.reshape(128, 72)
        cpak[:, C_MW:C_MW + 24] = mw.reshape(128, 24)
        pgx = np.zeros((7, PGW), np.float16)
        tiles = offd[c::8] + [None] * (36 - len(offd[c::8])) \
            + diag[c::8] + [None] * (4 - len(diag[c::8]))
        for t, bp in enumerate(tiles):
            sl = slice(t * 128, (t + 1) * 128)
            if bp is None:
                pgx[:, P_PA + t * 128:P_PA + (t + 1) * 128] = dumA
                pgx[:, P_PB + t * 128:P_PB + (t + 1) * 128] = dumB
                pgx[:, P_GA + t * 128:P_GA + (t + 1) * 128] = dumA
                pgx[:, P_GB + t * 128:P_GB + (t + 1) * 128] = dumB
                continue
            bi, bj = bp
            ra = slice(bi * 128, (bi + 1) * 128)
            rb = slice(bj * 128, (bj + 1) * 128)
            pgx[:, P_PA + t * 128:P_PA + (t + 1) * 128] = pA[:, ra]
            pgx[:, P_PB + t * 128:P_PB + (t + 1) * 128] = pB[:, rb]
            pgx[:, P_GA + t * 128:P_GA + (t + 1) * 128] = gA[:, ra]
            pgx[:, P_GB + t * 128:P_GB + (t + 1) * 128] = gB[:, rb]
        in_maps.append({"logits": lgc, "cpak": cpak, "pgx": pgx})
    return in_maps


def _host_combine(outs, inputs):
    tm = np.asarray(inputs["token_mask"][0], dtype=np.float64)
    ae = np.asarray(inputs["atom_exists"][0], dtype=np.float64)
    ts = float(np.asarray(inputs["timesteps"])[0, 0])

    errsum = 0.0
    num_sig = 0.0
    den_c = 0.0
    for o in outs:
        o = o.astype(np.float64)
        S = o[:, O_S:O_S + 576]
        errsum += np.log(S).sum()
        errsum -= o[:, O_B:O_B + 12].sum() + o[:, O_L0:O_L0 + 12].sum()
        num_sig += 2.0 * o[:, O_SIG:O_SIG + 4].sum() + o[:, O_SIG + 4:O_SIG + 8].sum()
        # O_CC cols hold sum(cbar) = 30 * #far per group; close = pairs - far
        npg = 128 * 512
        close = [npg - o[:, O_CC + g].sum() / 30.0 for g in range(10)]
        den_c += 2.0 * sum(close[0:9]) + close[9]

    denom = 1e-6 + tm.sum() ** 2
    l_disto = errsum / denom

    # diagonal removal: each atom contributes c=1 and sigmas at delta~0
    sig0 = sum(1.0 / (1.0 + np.exp(-(k))) for k in (0.5, 1.0, 2.0, 4.0))
    num = num_sig / 4.0 - NA * sig0 / 4.0
    den = den_c - NA
    lddt = num / (den + 1e-5)
    l_lddt = 1.0 - lddt

    # MSE from core 0's reductions
    m = outs[0][:, O_MSE:O_MSE + 18].astype(np.float64).sum(0)
    Sw, A3, B3 = m[0], m[1:4], m[4:7]
    M9 = m[7:16].reshape(3, 3)
    Spp, Sgg = m[16], m[17]
    wsum = Sw + 1e-5
    mu = A3 / wsum          # gt centroid
    mugt = B3 / wsum        # pred centroid
    H = M9 - np.outer(mugt, A3) - np.outer(B3, mu) + Sw * np.outer(mugt, mu)
    U, s, Vt = np.linalg.svd(H)
    d = np.sign(np.linalg.det(U @ Vt))
    tr = s[0] + s[1] + d * s[2]
    Swg = Spp - 2 * B3 @ mugt + Sw * (mugt @ mugt)
    Swc = Sgg - 2 * A3 @ mu + Sw * (mu @ mu)
    msesum = Swg + Swc - 2 * tr + 1e-5 * Sw
    mse = msesum / (1e-5 + ae.sum()) / 3.0
    scale = (ts ** 2 + 16.0 ** 2) / ((ts * 16.0) ** 2 + 1e-5)
    l_mse = scale * mse

    total = 0.03 * l_disto + 1.0 * l_lddt + 4.0 * l_mse
    return np.float32(total)


def _run(inputs, trace=False):
    from concourse.bass_utils import run_bass_kernel_spmd
    if "nc" not in _cache:
        _cache["nc"] = _build_graph()
    nc = _cache["nc"]
    in_maps = _host_prep(inputs)
    res = run_bass_kernel_spmd(nc, in_maps, list(range(NCORES)), trace=trace)
    outs = [res.results[c]["out"] for c in range(NCORES)]
    return _host_combine(outs, inputs), res


def _numpy_reference(inputs):
    """Exact reference in numpy; only used if masks are not all ones
    (never the case for this problem's setup_inputs)."""
    lg = np.asarray(inputs["distogram_logits"][0], np.float32)
    pos = np.asarray(inputs["all_atom_positions"][0], np.float32)
    tm = np.asarray(inputs["token_mask"][0], np.float32)
    pred = np.asarray(inputs["denoised_atoms"][0], np.float64)
    gt = np.asarray(inputs["augmented_gt_atoms"][0], np.float64)
    ts = float(np.asarray(inputs["timesteps"])[0, 0])
    ae = np.asarray(inputs["atom_exists"][0], np.float64)

    pb = pos[:, 1, :].astype(np.float64)
    d2 = ((pb[:, None] - pb[None, :]) ** 2).sum(-1)
    bounds = np.linspace(0.0, 32.0, 63) ** 2
    tb = (d2[:, :, None] > bounds).sum(-1)
    mx = lg.max(-1, keepdims=True)
    lse = np.log(np.exp(lg - mx).sum(-1)) + mx[..., 0]
    errors = lse - np.take_along_axis(lg, tb[:, :, None], -1)[..., 0]
    sqm = tm[:, None] * tm[None, :]
    l_disto = (errors * sqm).sum() / (1e-6 + sqm.sum())

    dp = np.sqrt(((pred[:, None] - pred[None, :]) ** 2).sum(-1) + 1e-6)
    dg = np.sqrt(((gt[:, None] - gt[None, :]) ** 2).sum(-1) + 1e-6)
    delta = np.abs(dg - dp)
    eps_lm = sum(1 / (1 + np.exp(-(k - delta))) for k in (0.5, 1.0, 2.0, 4.0)) / 4
    c = (dg < 15.0) * (ae[:, None] * ae[None, :]) * (1 - np.eye(NA))
    l_lddt = 1.0 - (eps_lm * c).sum() / (c.sum() + 1e-5)

    w = ae * ae
    wsum = w.sum() + 1e-5
    mu = (gt * w[:, None]).sum(0) / wsum
    mugt = (pred * w[:, None]).sum(0) / wsum
    xc, xgc = gt - mu, pred - mugt
    H = np.einsum('a,ai,aj->ij', w, xgc, xc)
    U, sv, Vt = np.linalg.svd(H)
    d = np.sign(np.linalg.det(U @ Vt))
    U[:, -1] *= d
    R = U @ Vt
    aligned = xc @ R.T + mugt
    atom_mse = (((pred - aligned) ** 2).sum(-1) + 1e-5) * ae * ae
    mse = atom_mse.sum() / (1e-5 + ae.sum()) / 3.0
    scale = (ts ** 2 + 256.0) / ((ts * 16.0) ** 2 + 1e-5)
    return np.float32(0.03 * l_disto + l_lddt + 4.0 * scale * mse)


def kernel(**inputs):
    tm = np.asarray(inputs["token_mask"])
    ae = np.asarray(inputs["atom_exists"])
    if not (np.all(tm == 1.0) and np.all(ae == 1.0)):
        return _numpy_reference(inputs)
    out, _ = _run(inputs, trace=False)
    return out


def kernel_traced(**inputs):
    return _run(inputs, trace=True)



# revision 11
# speedup vs baseline: 1.9844x; 1.9844x over previous
"""AlphaFold3Loss Bass kernel for 8 TRN2 NeuronCores.

v2 design. Device does the O(N^2) memory/compute-heavy streaming; host
does exact scalar bookkeeping it can compute from its own copy of the
inputs (it already holds them):

Distogram (device): per pair, S = sum_b exp(L_b) via a DVE fast-exp —
  tensor_scalar(L*1024/ln2 + (15360-C)) with int16 output rounds to the
  fp16 bit pattern of exp(L) (round-to-nearest verified on HW; C=59
  calibrated so the lse bias is ~0, per-pair sd 3.5e-3 averages out
  over 590k pairs) — then a 5-level fp16 pairwise tree sum. Logits are
  DMA'd as bf16 (halves HBM traffic; lse err from bf16 logits is
  zero-mean, ~4e-3 per pair). ACT is not used for the distogram at all.
Distogram (host): errsum = sum log S (device S) - sum_pairs L_tb, with
  the true-bin gather take_along_axis-style from the host's f32 logits.
LDDT (device): d2 via PE K=7 fp16 augmented matmuls; sqrt on ACT (f16
  out); delta/cbar/dpr on DVE in f16 (2x/4x modes; abs_max fuses
  abs+max); far-pair cutoff via delta' = max(|dp-dg|, 30*(dg>=15)) so
  sigmoid terms of cutoff pairs vanish; close-pair counts accumulated
  on PE (ones^T @ cbar into PSUM, off-diag and diag separately);
  4 sigmoid passes on ACT with accum_out. Diagonal pairs removed on
  host.
MSE: entirely on host (f64, exact) — O(NA) reductions + 3x3 SVD.

Sharding: distogram rows 768 -> 96/core; LDDT 128-atom block pairs
dealt round-robin (36 off-diag + 4 diag tiles/core, symmetric blocks
counted once and doubled on host).

Assumes token_mask/atom_exists are all ones (true for setup_inputs);
otherwise kernel() falls back to an exact numpy path.
"""
import sys
sys.path.insert(0, '/opt/trn_rl_repo')
import numpy as np
import ml_dtypes
from contextlib import ExitStack

NT, NO_BINS, NA = 768, 64, 3072
NCORES = 8
RPC = NT // NCORES          # 96 distogram rows per core
NMT = RPC // 8              # 12 mega-tiles (8 rows each)
D_EPS = 4e-3                # lddt sqrt guard (host aug7 adds it)
BIG = 30.0
FEXP_A = 1024.0 / np.log(2.0)
FEXP_B = 15360.0 - 59.0     # C=59 calibrated for ~zero lse bias

# pgx column layout (partition dim = 7, fp16): A/B aug forms, 40 tiles x 128
NTIL = 40                   # 36 offdiag (w=2) + 4 diag (w=1) lddt tiles/core
P_PA, P_PB, P_GA, P_GB = 0, 5120, 10240, 15360
PGW = 20480
# out column layout
O_S = 0                     # 576 per-pair sum-exp
O_SIG = 576                 # 8 sigmoid accums (4 k x {offdiag, diag})
O_CCO = 584                 # sum(cbar) over offdiag groups (partition 0)
O_CCD = 585                 # sum(cbar) over diag group (partition 0)
OW = 640

_cache = {}


def _sched(name):
    """Emission order: ('g', i) lddt group, ('m', i) disto mega-tile,
    ('r', 0) count reduces."""
    if name == "A":      # all lddt groups, reduces, then disto
        return [("g", i) for i in range(10)] + [("r", 0)] \
            + [("m", i) for i in range(NMT)]
    if name == "B":      # 1:1 interleave
        s = []
        for i in range(NMT):
            if i < 10:
                s.append(("g", i))
            s.append(("m", i))
        return s + [("r", 0)]
    if name == "C":      # 2 groups then 1 mt
        s = []
        for i in range(5):
            s += [("g", 2 * i), ("g", 2 * i + 1), ("m", i)]
        return s + [("r", 0)] + [("m", i) for i in range(5, NMT)]
    if name == "D":      # prime with one mt, then lddt block, rest disto
        return [("m", 0)] + [("g", i) for i in range(10)] + [("r", 0)] \
            + [("m", i) for i in range(1, NMT)]
    raise ValueError(name)


def _build_graph(phases=("disto", "lddt"), reps=1, fastexp=True, sched_name="A"):
    sched = _sched(sched_name)
    from concourse import bass, bacc, tile, mybir
    F32 = mybir.dt.float32
    F16 = mybir.dt.float16
    BF16 = mybir.dt.bfloat16
    I16 = mybir.dt.int16
    U16 = mybir.dt.uint16
    AF = mybir.ActivationFunctionType
    ALU = mybir.AluOpType
    AX = mybir.AxisListType

    from concourse.tile import add_dep_helper
    nc = bacc.Bacc(None, target_bir_lowering=False)
    LDT = BF16 if fastexp else F32
    lg_ext = nc.declare_dram_parameter("logits", [128, RPC, 384], LDT, isOutput=False)
    cb_ext = nc.declare_dram_parameter("cb", [128, 8], F32, isOutput=False)
    px_ext = nc.declare_dram_parameter("pgx", [7, PGW], F16, isOutput=False)
    out_ext = nc.declare_dram_parameter("out", [128, OW], F32, isOutput=True)

    with tile.TileContext(nc) as tc, ExitStack() as ctx:
        const = ctx.enter_context(tc.tile_pool(name="const", bufs=1))
        lpool = ctx.enter_context(tc.tile_pool(name="lp", bufs=4))
        epool = ctx.enter_context(tc.tile_pool(name="ep", bufs=2))
        spool = ctx.enter_context(tc.tile_pool(name="sp", bufs=2))
        psum = ctx.enter_context(tc.tile_pool(name="ps", bufs=1, space="PSUM"))

        pgx = const.tile([7, PGW], F16)
        nc.sync.dma_start(pgx[:], px_ext[:, :])
        cb = const.tile([128, 8], F32)
        nc.sync.dma_start(cb[:], cb_ext[:, :])
        outb = const.tile([128, OW], F32)
        nc.vector.memset(outb[:], 0.0)
        if reps > 1:
            racc = const.tile([128, OW], F32)
            nc.vector.memset(racc[:], 0.0)
        dpr = const.tile([128, 10 * 512], F16)   # stored delta' for phase B
        ones128 = const.tile([128, 1], F16)
        nc.vector.memset(ones128[:], 1.0)
        if "lddt" in phases:
            # pull the Sqrt act-table load off the first lddt group's
            # critical path: load it at t=0 while pgx is still in flight
            warm = const.tile([128, 1], F32)
            nc.scalar.activation(warm[:], ones128[:], AF.Sqrt)

        sqrt_insts, exp_insts, sig_insts = [], [], []

        for _rep in range(reps):
            def emit_lddt_group(g10):
                psP = psum.tile([128, 512], F32, tag="psP", bufs=2)
                psG = psum.tile([128, 512], F32, tag="psG", bufs=2)
                for q in range(4):
                    t = g10 * 4 + q
                    nc.tensor.matmul(psP[:, q * 128:(q + 1) * 128],
                                     lhsT=pgx[:, P_PA + t * 128:P_PA + (t + 1) * 128],
                                     rhs=pgx[:, P_PB + t * 128:P_PB + (t + 1) * 128],
                                     start=True, stop=True)
                    nc.tensor.matmul(psG[:, q * 128:(q + 1) * 128],
                                     lhsT=pgx[:, P_GA + t * 128:P_GA + (t + 1) * 128],
                                     rhs=pgx[:, P_GB + t * 128:P_GB + (t + 1) * 128],
                                     start=True, stop=True)
                dp = spool.tile([128, 512], F16, tag="dp")
                sqrt_insts.append(nc.scalar.activation(dp[:], psP[:], AF.Sqrt))
                dg = spool.tile([128, 512], F16, tag="dg")
                sqrt_insts.append(nc.scalar.activation(dg[:], psG[:], AF.Sqrt))
                delta = spool.tile([128, 512], F16, tag="delta")
                nc.vector.tensor_sub(delta[:], dp[:], dg[:])
                nc.vector.tensor_scalar(delta[:].bitcast(U16), delta[:].bitcast(U16),
                                        0x7FFF, None, ALU.bitwise_and)
                cbar = spool.tile([128, 512], F16, tag="cbar")
                nc.vector.tensor_scalar(cbar[:], dg[:], 15.0, BIG, ALU.is_ge, ALU.mult)
                nc.vector.tensor_tensor(dpr[:, g10 * 512:(g10 + 1) * 512],
                                        delta[:], cbar[:], ALU.max)
                cc = ccD if g10 == 9 else ccO
                nc.tensor.matmul(cc[:], lhsT=ones128[:], rhs=cbar[:],
                                 start=(g10 in (0, 9)), stop=(g10 in (8, 9)),
                                 skip_group_check=True)

            def emit_disto_mt(mt):
                L = lpool.tile([128, 8 * 384], LDT, tag="L")
                nc.sync.dma_start(L[:], lg_ext[:, 8 * mt:8 * mt + 8, :])
                if "dmaraw" in phases:
                    nc.vector.tensor_reduce(outb[:, O_S + mt:O_S + mt + 1],
                                            L[:, 0:64], AX.X, ALU.add)
                    return
                if fastexp:
                    Ei = epool.tile([128, 8 * 384], I16, tag="E")
                    nc.vector.tensor_scalar(Ei[:], L[:], FEXP_A, FEXP_B,
                                            ALU.mult, ALU.add)
                    E3 = Ei[:].bitcast(F16).rearrange("p (a b) -> p a b", b=64)
                else:
                    E = epool.tile([128, 8 * 384], BF16, tag="E")
                    exp_insts.append(nc.scalar.activation(E[:], L[:], AF.Exp))
                    E3 = E[:].rearrange("p (a b) -> p a b", b=64)
                w = 32
                while w >= 2:
                    nc.vector.tensor_add(E3[:, :, 0:w], E3[:, :, 0:w], E3[:, :, w:2 * w])
                    w //= 2
                nc.vector.tensor_add(outb[:, O_S + mt * 48:O_S + (mt + 1) * 48],
                                     E3[:, :, 0], E3[:, :, 1])

            do_lddt = "lddt" in phases
            do_disto = "disto" in phases or "dmaraw" in phases
            if do_lddt:
                ccO = psum.tile([1, 512], F32, tag="ccO", bufs=1)
                ccD = psum.tile([1, 512], F32, tag="ccD", bufs=1)
            for tok, i in sched:
                if tok == "g" and do_lddt:
                    emit_lddt_group(i)
                elif tok == "m" and do_disto:
                    emit_disto_mt(i)
                elif tok == "r" and do_lddt:
                    nc.vector.tensor_reduce(outb[0:1, O_CCO:O_CCO + 1], ccO[:], AX.X, ALU.add)
                    nc.vector.tensor_reduce(outb[0:1, O_CCD:O_CCD + 1], ccD[:], AX.X, ALU.add)

            # ---------------- LDDT phase B: sigmoid sums ---------------------
            if "lddt" in phases:
                for part, lo, hi in ((0, 0, 9 * 512), (1, 9 * 512, 10 * 512)):
                    for k in range(4):
                        sg = spool.tile([128, 9 * 512], F16, tag="sg")
                        col = O_SIG + part * 4 + k
                        sig_insts.append(nc.scalar.activation(
                            sg[:, 0:hi - lo], dpr[:, lo:hi], AF.Sigmoid,
                            bias=cb[:, k:k + 1], scale=-1.0,
                            accum_out=outb[:, col:col + 1]))

            if reps > 1:
                nc.vector.tensor_add(racc[:], racc[:], outb[:])

        # ACT table-set ordering: sqrts -> exps -> sigmoids
        if sqrt_insts and exp_insts:
            add_dep_helper(sqrt_insts[-1].ins, exp_insts[0].ins, sync=False,
                           reason="act table: sqrts before exps")
        if exp_insts and sig_insts:
            add_dep_helper(exp_insts[-1].ins, sig_insts[0].ins, sync=False,
                           reason="act table: exps before sigmoids")
        elif sqrt_insts and sig_insts:
            add_dep_helper(sqrt_insts[-1].ins, sig_insts[0].ins, sync=False,
                           reason="act table: sqrts before sigmoids")

        nc.sync.dma_start(out_ext[:, :], racc[:] if reps > 1 else outb[:])
    nc.finalize()
    return nc


def _host_prep(inputs, fastexp=True):
    lg = np.ascontiguousarray(inputs["distogram_logits"][0], dtype=np.float32)  # [768,768,64]
    pred = np.asarray(inputs["denoised_atoms"][0], dtype=np.float32)            # [3072,3]
    gt = np.asarray(inputs["augmented_gt_atoms"][0], dtype=np.float32)

    def aug7(x):
        """fp16 K=7 aug: A (stationary) and B (moving) forms per atom.
        d2 = -2<xq,yq> + (hi_m+lo_m) + (hi_n+lo_n); rn from the fp16-quantized
        coords, hi/lo split so fp16 carries rn to ~1e-3 abs."""
        xq = x.astype(np.float16).astype(np.float64)
        rn = (xq ** 2).sum(-1)
        hi = rn.astype(np.float16)
        lo = rn - hi.astype(np.float64)
        one = np.ones(len(x))
        A = np.stack([-2 * xq[:, 0], -2 * xq[:, 1], -2 * xq[:, 2],
                      hi.astype(np.float64), lo + D_EPS, one, one]).astype(np.float16)
        B = np.stack([xq[:, 0], xq[:, 1], xq[:, 2], one, one,
                      hi.astype(np.float64), lo]).astype(np.float16)
        return A, B

    pA, pB = aug7(pred)
    gA, gB = aug7(gt)
    dumA = np.zeros((7, 128), np.float16)
    dumA[3] = 6.0e4; dumA[5] = 1.0; dumA[6] = 1.0
    dumB = np.zeros((7, 128), np.float16)
    dumB[3] = 1.0; dumB[4] = 1.0; dumB[5] = 6.0e4
    # symmetric block-tile assignment: 24 atom blocks of 128
    offd = [(i, j) for i in range(24) for j in range(i + 1, 24)]
    diag = [(i, i) for i in range(24)]

    cb = np.zeros((128, 8), np.float32)
    cb[:, 0:4] = np.array([0.5, 1.0, 2.0, 4.0], np.float32)

    in_maps = []
    for c in range(NCORES):
        rows = slice(RPC * c, RPC * (c + 1))
        lgc = lg[rows].reshape(RPC, 128, 384).transpose(1, 0, 2)
        if fastexp:
            lgc = lgc.astype(ml_dtypes.bfloat16)
        else:
            lgc = np.ascontiguousarray(lgc)
        pgx = np.zeros((7, PGW), np.float16)
        tiles = offd[c::8] + [None] * (36 - len(offd[c::8])) \
            + diag[c::8] + [None] * (4 - len(diag[c::8]))
        for t, bp in enumerate(tiles):
            if bp is None:
                pgx[:, P_PA + t * 128:P_PA + (t + 1) * 128] = dumA
                pgx[:, P_PB + t * 128:P_PB + (t + 1) * 128] = dumB
                pgx[:, P_GA + t * 128:P_GA + (t + 1) * 128] = dumA
                pgx[:, P_GB + t * 128:P_GB + (t + 1) * 128] = dumB
                continue
            bi, bj = bp
            ra = slice(bi * 128, (bi + 1) * 128)
            rb = slice(bj * 128, (bj + 1) * 128)
            pgx[:, P_PA + t * 128:P_PA + (t + 1) * 128] = pA[:, ra]
            pgx[:, P_PB + t * 128:P_PB + (t + 1) * 128] = pB[:, rb]
            pgx[:, P_GA + t * 128:P_GA + (t + 1) * 128] = gA[:, ra]
            pgx[:, P_GB + t * 128:P_GB + (t + 1) * 128] = gB[:, rb]
        in_maps.append({"logits": lgc, "cb": cb, "pgx": pgx})
    return in_maps


def _host_combine(outs, inputs):
    lg = np.asarray(inputs["distogram_logits"][0], np.float32)
    pos = np.asarray(inputs["all_atom_positions"][0], np.float32)
    tm = np.asarray(inputs["token_mask"][0], np.float64)
    ae = np.asarray(inputs["atom_exists"][0], np.float64)
    ts = float(np.asarray(inputs["timesteps"])[0, 0])

    # ---- distogram: device S; host true-bin gather (exact f32 like ref) ----
    pb = pos[:, 1, :]                                   # CA positions [768,3]
    d2 = ((pb[:, None, :] - pb[None, :, :]) ** 2).sum(-1)      # f32 [768,768]
    bounds = (np.linspace(0.0, 32.0, 63).astype(np.float32)) ** 2
    tb = np.searchsorted(bounds, d2.ravel(), side="left")
    Ltb = lg.reshape(-1, NO_BINS)[np.arange(tb.size), tb]
    errsum = -Ltb.astype(np.float64).sum()
    for o in outs:
        errsum += np.log(o[:, O_S:O_S + 576].astype(np.float64)).sum()
    denom = 1e-6 + tm.sum() ** 2
    l_disto = errsum / denom

    # ---- lddt ----
    num_sig = 0.0
    den_c = 0.0
    for o in outs:
        o64 = o.astype(np.float64)
        num_sig += 2.0 * o64[:, O_SIG:O_SIG + 4].sum() \
            + o64[:, O_SIG + 4:O_SIG + 8].sum()
        close_off = 9 * 128 * 512 - o64[0, O_CCO] / BIG
        close_diag = 128 * 512 - o64[0, O_CCD] / BIG
        den_c += 2.0 * close_off + close_diag
    # diagonal removal: each atom contributes c=1 and sigmas at delta~0
    sig0 = sum(1.0 / (1.0 + np.exp(-k)) for k in (0.5, 1.0, 2.0, 4.0))
    num = num_sig / 4.0 - NA * sig0 / 4.0
    den = den_c - NA
    l_lddt = 1.0 - num / (den + 1e-5)

    # ---- mse (host, f64 exact) ----
    pred = np.asarray(inputs["denoised_atoms"][0], np.float64)
    gt = np.asarray(inputs["augmented_gt_atoms"][0], np.float64)
    w = ae * ae
    wsum = w.sum() + 1e-5
    mu = (gt * w[:, None]).sum(0) / wsum
    mugt = (pred * w[:, None]).sum(0) / wsum
    xc, xgc = gt - mu, pred - mugt
    H = np.einsum('a,ai,aj->ij', w, xgc, xc)
    U, sv, Vt = np.linalg.svd(H)
    d = np.sign(np.linalg.det(U @ Vt))
    U[:, -1] *= d
    R = U @ Vt
    aligned = xc @ R.T + mugt
    atom_mse = (((pred - aligned) ** 2).sum(-1) + 1e-5) * ae * ae
    mse = atom_mse.sum() / (1e-5 + ae.sum()) / 3.0
    scale = (ts ** 2 + 256.0) / ((ts * 16.0) ** 2 + 1e-5)
    l_mse = scale * mse

    total = 0.03 * l_disto + 1.0 * l_lddt + 4.0 * l_mse
    return np.float32(total)


def _run(inputs, trace=False):
    from concourse.bass_utils import run_bass_kernel_spmd
    if "nc" not in _cache:
        _cache["nc"] = _build_graph()
    nc = _cache["nc"]
    in_maps = _host_prep(inputs)
    res = run_bass_kernel_spmd(nc, in_maps, list(range(NCORES)), trace=trace)
    outs = [res.results[c]["out"] for c in range(NCORES)]
    return _host_combine(outs, inputs), res


def _numpy_reference(inputs):
    """Exact reference in numpy; only used if masks are not all ones
    (never the case for this problem's setup_inputs)."""
    lg = np.asarray(inputs["distogram_logits"][0], np.float32)
    pos = np.asarray(inputs["all_atom_positions"][0], np.float32)
    tm = np.asarray(inputs["token_mask"][0], np.float32)
    pred = np.asarray(inputs["denoised_atoms"][0], np.float64)
    gt = np.asarray(inputs["augmented_gt_atoms"][0], np.float64)
    ts = float(np.asarray(inputs["timesteps"])[0, 0])
    ae = np.asarray(inputs["atom_exists"][0], np.float64)

    pb = pos[:, 1, :].astype(np.float64)
    d2 = ((pb[:, None] - pb[None, :]) ** 2).sum(-1)
    bounds = np.linspace(0.0, 32.0, 63) ** 2
    tb = (d2[:, :, None] > bounds).sum(-1)
    mx = lg.max(-1, keepdims=True)
    lse = np.log(np.exp(lg - mx).sum(-1)) + mx[..., 0]
    errors = lse - np.take_along_axis(lg, tb[:, :, None], -1)[..., 0]
    sqm = tm[:, None] * tm[None, :]
    l_disto = (errors * sqm).sum() / (1e-6 + sqm.sum())

    dp = np.sqrt(((pred[:, None] - pred[None, :]) ** 2).sum(-1) + 1e-6)
    dg = np.sqrt(((gt[:, None] - gt[None, :]) ** 2).sum(-1) + 1e-6)
    delta = np.abs(dg - dp)
    eps_lm = sum(1 / (1 + np.exp(-(k - delta))) for k in (0.5, 1.0, 2.0, 4.0)) / 4
    c = (dg < 15.0) * (ae[:, None] * ae[None, :]) * (1 - np.eye(NA))
    l_lddt = 1.0 - (eps_lm * c).sum() / (c.sum() + 1e-5)

    w = ae * ae
    wsum = w.sum() + 1e-5
    mu = (gt * w[:, None]).sum(0) / wsum
    mugt = (pred * w[:, None]).sum(0) / wsum
    xc, xgc = gt - mu, pred - mugt
    H = np.einsum('a,ai,aj->ij', w, xgc, xc)
    U, sv, Vt = np.linalg.svd(H)
    d = np.sign(np.linalg.det(U @ Vt))
    U[:, -1] *= d
    R = U @ Vt
    aligned = xc @ R.T + mugt
    atom_mse = (((pred - aligned) ** 2).sum(-1) + 1e-5) * ae * ae
    mse = atom_mse.sum() / (1e-5 + ae.sum()) / 3.0
    scale = (ts ** 2 + 256.0) / ((ts * 16.0) ** 2 + 1e-5)
    return np.float32(0.03 * l_disto + l_lddt + 4.0 * scale * mse)


def kernel(**inputs):
    tm = np.asarray(inputs["token_mask"])
    ae = np.asarray(inputs["atom_exists"])
    if not (np.all(tm == 1.0) and np.all(ae == 1.0)):
        return _numpy_reference(inputs)
    out, _ = _run(inputs, trace=False)
    return out


def kernel_traced(**inputs):
    return _run(inputs, trace=True)
